# revision 13
# baseline (speedup 1.0000x reference)
"""Cross-attention Trainium2 kernel (Bass/Tile), data-parallel over batch.

B=8 batch elements -> 8 NeuronCores, one batch element per core.
Per core: y = softmax(q Wq (kv Wk)^T / sqrt(dk)) (kv Wv) Wo + bo
with S1=S2=2048, D=1024, H=8, DK=DV=128.

Layout strategy (everything bf16 on the PE, fp32 softmax stats):
  - inputs are cast fp32->bf16 during the SWDGE load, then PE-transposed
    to qT/kvT [D, S] tiles.  Input-row DMAs are issued BEFORE the weight
    DMAs so the PE starts within a few us instead of waiting ~50us for
    16MB of weights to stream in.
  - projections produce QT,KT  [H*DK, S] (head-major partition chunks) and
    V [S2, H*DV] (natural), all bf16 in SBUF.
  - scores S = QT_h^T KT_h computed transposed per 2x128-row groups into
    PSUM (2 banks x bufs=2 so PE runs ahead of ACT), exp on ACT (no max
    subtraction: |s*scale| < ~3), giving PT [s2, 16, s1-blk] bf16.
  - softmax denominators: 4-level DVE tree-add over PT chunks (bf16 2x
    mode) + ONE ones-matmul on PE (instead of 16 full-stream matmuls),
    reciprocal computed in a [128,4] layout (128 lanes instead of 1),
    broadcast via a small DRAM bounce.
  - O^T = sum_c V_c^T PT_c per head; normalized on DVE; the output
    projection consumes O^T directly with Wo natural.
  - bias bo is pre-broadcast to [128, D] once and added on DVE during the
    PSUM drain of the output projection (no ones-matmuls).
"""

import os

import numpy as np

import concourse.bass as bass
import concourse.mybir as mybir
import concourse.tile as tile
from concourse import bacc
from concourse.bass_utils import run_bass_kernel_spmd
from concourse.masks import make_identity

B = 8
S = 2048  # S1 == S2
D = 1024  # D1 == D2
H = 8
DK = DV = 128
KC = D // 128  # contraction chunks
SC = S // 128  # sequence chunks of 128
BLK = 512
NBLK = S // BLK
SCALE = 1.0 / float(np.sqrt(DK))

F32 = mybir.dt.float32
BF16 = mybir.dt.bfloat16
EXP = mybir.ActivationFunctionType.Exp


def _emit(tc, aps):
    nc = tc.nc
    query, key_value, Wq, Wk, Wv, Wo, bo, out = (
        aps["query"], aps["key_value"], aps["Wq"], aps["Wk"], aps["Wv"],
        aps["Wo"], aps["bo"], aps["out"],
    )

    persist = tc.alloc_tile_pool(name="persist", bufs=1)
    QT_sb = persist.tile([128, H, S], BF16, name="QT_sb")
    KT_sb = persist.tile([128, H, S], BF16, name="KT_sb")
    V_sb = persist.tile([128, SC, H * DV], BF16, name="V_sb")
    Wo_sb = persist.tile([128, KC, D], BF16, name="Wo_sb")
    bo_bc = persist.tile([128, D], F32, name="bo_bc")
    onec_sb = persist.tile([128, 1], BF16, name="onec_sb")

    ident = persist.tile([128, 128], BF16, name="ident")
    make_identity(nc, ident)
    nc.vector.memset(onec_sb, 1.0)
    # bias broadcast to all 128 partitions (tiny, one-time)
    bo_bcast = bass.AP(
        tensor=bo.tensor, offset=bo.offset, ap=[[0, 128]] + list(bo.ap[1:])
    )
    nc.sync.dma_start(out=bo_bc, in_=bo_bcast)

    def load_weight(dst, src):
        # split the cast-DMA per 128-row chunk so dependent matmuls can
        # start as soon as their contraction chunk lands
        srcv = src.rearrange("(kc p) n -> p kc n", p=128)
        for kc in range(KC):
            nc.gpsimd.dma_start(out=dst[:, kc, :], in_=srcv[:, kc, :])

    def pe_transpose8(tpool, dst8, src):
        """Transpose eight [128,128] bf16 tiles of src through one PSUM
        bank and copy into dst8 [128, 8, 128] on DVE."""
        tp = tpool.tile([128, 1024], BF16, name="tp", tag="tp")
        for i in range(KC):
            nc.tensor.transpose(
                tp[:, i * 128:(i + 1) * 128], src[:, i * 128:(i + 1) * 128],
                ident,
            )
        nc.vector.tensor_copy(dst8, tp.rearrange("p (c f) -> p c f", c=8))

    def load_transposed_block(work, tpool, src_ap, j, tag):
        """Load 512 rows of src [S, D] f32, return xT block [128, KC, 512]."""
        xT = work.tile([128, KC, BLK], BF16, name=f"{tag}T", tag=f"{tag}T",
                       bufs=2)
        for c4 in range(4):
            c = j * 4 + c4
            row = work.tile([128, D], BF16, name=f"{tag}row", tag="row",
                            bufs=3)
            nc.gpsimd.dma_start(out=row, in_=src_ap[c * 128:(c + 1) * 128, :])
            pe_transpose8(tpool, xT[:, :, c4 * 128:(c4 + 1) * 128], row)
        return xT

    # weight tiles allocated up-front so their DMAs can be staggered
    # between the input-row DMAs inside the kv loop (LIFO pool order)
    wqp = tc.alloc_tile_pool(name="wqp", bufs=1)
    Wq_sb = wqp.tile([128, KC, D], BF16, name="Wq_sb")
    wkv = tc.alloc_tile_pool(name="wkv", bufs=1)
    Wk_sb = wkv.tile([128, KC, D], BF16, name="Wk_sb")
    Wv_sb = wkv.tile([128, KC, D], BF16, name="Wv_sb")

    # ---- phase 1a: K/V projections --------------------------------------
    with nc.named_scope("ph1_kv"), \
         tc.tile_pool(name="p1work_kv", bufs=1) as work, \
         tc.tile_pool(name="p1tp_kv", bufs=2, space="PSUM") as tp1, \
         tc.tile_pool(name="p1psum_kv", bufs=4, space="PSUM") as pps:
        for j in range(NBLK):
            kvT = load_transposed_block(work, tp1, key_value, j, "kv")
            if j == 0:
                load_weight(Wk_sb, Wk)
                load_weight(Wv_sb, Wv)
            elif j == 1:
                load_weight(Wq_sb, Wq)
            elif j == 2:
                load_weight(Wo_sb, Wo)
            # KT block: out[M=dk chunk m (head), N=s2] += Wk[kc,m].T @ kvT[kc]
            for m in range(H):
                ps = pps.tile([128, BLK], F32, name="ps_k", tag="pps")
                for kc in range(KC):
                    nc.tensor.matmul(
                        ps, lhsT=Wk_sb[:, kc, m * 128:(m + 1) * 128],
                        rhs=kvT[:, kc, :], start=(kc == 0), stop=(kc == KC - 1),
                    )
                nc.scalar.copy(KT_sb[:, m, j * BLK:(j + 1) * BLK], ps)
            # V block rows: out[M=s2 sub, N=hdv] += kvT[kc, sub].T @ Wv[kc]
            for m4 in range(4):
                for n in range(2):
                    ps = pps.tile([128, BLK], F32, name="ps_v", tag="pps")
                    for kc in range(KC):
                        nc.tensor.matmul(
                            ps, lhsT=kvT[:, kc, m4 * 128:(m4 + 1) * 128],
                            rhs=Wv_sb[:, kc, n * BLK:(n + 1) * BLK],
                            start=(kc == 0), stop=(kc == KC - 1),
                        )
                    nc.scalar.copy(
                        V_sb[:, j * 4 + m4, n * BLK:(n + 1) * BLK], ps
                    )
    wkv.release()

    # ---- phase 1b: Q projection -----------------------------------------
    with nc.named_scope("ph1_q"), \
         tc.tile_pool(name="p1work_q", bufs=1) as work, \
         tc.tile_pool(name="p1tp_q", bufs=2, space="PSUM") as tp1, \
         tc.tile_pool(name="p1psum_q", bufs=4, space="PSUM") as pps:
        for j in range(NBLK):
            qT = load_transposed_block(work, tp1, query, j, "q")
            for m in range(H):
                ps = pps.tile([128, BLK], F32, name="ps_q", tag="pps")
                for kc in range(KC):
                    nc.tensor.matmul(
                        ps, lhsT=Wq_sb[:, kc, m * 128:(m + 1) * 128],
                        rhs=qT[:, kc, :], start=(kc == 0), stop=(kc == KC - 1),
                    )
                nc.scalar.copy(QT_sb[:, m, j * BLK:(j + 1) * BLK], ps)
    wqp.release()

    # ---- phase 2+3: attention + output projection -----------------------
    with nc.named_scope("attn"), \
         tc.tile_pool(name="p2", bufs=1) as p2, \
         tc.tile_pool(name="small", bufs=1) as small, \
         tc.tile_pool(name="spsum", bufs=2, space="PSUM") as spsum, \
         tc.tile_pool(name="supsum", bufs=1, space="PSUM") as supsum, \
         tc.tile_pool(name="opsum", bufs=2, space="PSUM") as opsum, \
         tc.tile_pool(name="ypsum", bufs=1, space="PSUM") as ypsum, \
         tc.tile_pool(name="dram", bufs=4, space="DRAM") as dpool:
        for j in range(NBLK):
            OT_sb = p2.tile([128, H, BLK], BF16, name="OT_sb", tag="OT", bufs=2)
            jcols = slice(j * BLK, (j + 1) * BLK)
            for h in range(H):
                # scores transposed: PT[c][s2_local, s1] = exp(K_h^T q)
                PT_sb = p2.tile([128, SC, BLK], BF16, name="PT_sb", tag="PT",
                                bufs=2)
                qblk = QT_sb[:, h, jcols]
                for g in range(8):
                    sps = spsum.tile([128, 2, BLK], F32, name="sps", tag="sps")
                    for i in range(2):
                        c = 2 * g + i
                        nc.tensor.matmul(
                            sps[:, i, :],
                            lhsT=KT_sb[:, h, c * 128:(c + 1) * 128],
                            rhs=qblk, start=True, stop=True,
                        )
                    nc.scalar.activation(
                        PT_sb[:, 2 * g:2 * (g + 1), :], sps, EXP, scale=SCALE,
                    )
                # weighted values O^T (unnormalized)
                ops = opsum.tile([128, BLK], F32, name="ops", tag="ops")
                for c in range(SC):
                    nc.tensor.matmul(
                        ops, lhsT=V_sb[:, c, h * 128:(h + 1) * 128],
                        rhs=PT_sb[:, c, :], start=(c == 0), stop=(c == SC - 1),
                    )
                # row sums over s2: DVE tree-add 16->4 chunks, then 4 small
                # ones-matmuls (instead of 16 full-stream PE matmuls).
                # Balanced pairing: the first half-tree only needs chunks
                # 0..7, so it runs while ACT is still producing 8..15 and
                # the post-exp tail is one level shorter.
                T1a = p2.tile([128, SC // 4, BLK], BF16, name="T1a", tag="T1a",
                              bufs=1)
                nc.vector.tensor_add(T1a, PT_sb[:, 0:4, :], PT_sb[:, 4:8, :])
                T1b = p2.tile([128, SC // 4, BLK], BF16, name="T1b", tag="T1b",
                              bufs=1)
                nc.vector.tensor_add(T1b, PT_sb[:, 8:12, :], PT_sb[:, 12:16, :])
                Rr = p2.tile([128, SC // 4, BLK], BF16, name="Rr", tag="Rr",
                             bufs=2)
                nc.vector.tensor_add(Rr, T1a, T1b)
                sus = supsum.tile([1, BLK], F32, name="sus", tag="sus")
                for c in range(4):
                    nc.tensor.matmul(
                        sus, lhsT=onec_sb, rhs=Rr[:, c, :],
                        start=(c == 0), stop=(c == 3),
                    )
                # reciprocal in [128, 4] layout (all lanes) via DRAM bounce,
                # then broadcast to [128, 512]
                s_row = small.tile([1, BLK], F32, name="s_row", tag="srow",
                                   bufs=2)
                nc.vector.tensor_copy(s_row, sus)
                ds = dpool.tile([1, BLK], F32, name="ds", tag="ds")
                nc.sync.dma_start(out=ds, in_=s_row)
                rec_in = small.tile([128, 4], F32, name="rec_in", tag="ri",
                                    bufs=2)
                nc.sync.dma_start(
                    out=rec_in, in_=ds.rearrange("o (p f) -> (o p) f", p=128)
                )
                rec4 = small.tile([128, 4], F32, name="rec4", tag="r4", bufs=2)
                nc.vector.reciprocal(rec4, rec_in)
                dr = dpool.tile([1, BLK], F32, name="dr", tag="dr")
                nc.sync.dma_start(
                    out=dr.rearrange("o (p f) -> (o p) f", p=128), in_=rec4
                )
                bc_sb = small.tile([128, BLK], F32, name="bc_sb", tag="bc",
                                   bufs=2)
                rec_bcast = bass.AP(
                    tensor=dr.tensor, offset=dr.offset,
                    ap=[[0, 128]] + list(dr.ap[1:]),
                )
                nc.gpsimd.dma_start(out=bc_sb, in_=rec_bcast)
                nc.vector.tensor_mul(OT_sb[:, h, :], ops, bc_sb)
            # output projection for block j (bias added on DVE)
            for m in range(4):
                for n in range(2):
                    yps = ypsum.tile([128, BLK], F32, name="yps", tag="yps")
                    for h8 in range(H):
                        nc.tensor.matmul(
                            yps, lhsT=OT_sb[:, h8, m * 128:(m + 1) * 128],
                            rhs=Wo_sb[:, h8, n * BLK:(n + 1) * BLK],
                            start=(h8 == 0), stop=(h8 == H - 1),
                        )
                    y_sb = p2.tile([128, BLK], F32, name="y_sb", tag="y",
                                   bufs=3)
                    nc.vector.tensor_add(
                        y_sb, yps, bo_bc[:, n * BLK:(n + 1) * BLK]
                    )
                    r0 = j * BLK + m * 128
                    nc.sync.dma_start(
                        out=out[r0:r0 + 128, n * BLK:(n + 1) * BLK], in_=y_sb
                    )
    persist.release()


_CACHE = {}


def _build():
    if "nc" in _CACHE:
        return _CACHE["nc"]
    nc = bacc.Bacc(
        "TRN2", target_bir_lowering=False, debug=False,
        enable_asserts=False, num_devices=B,
    )
    aps = {
        "query": nc.dram_tensor("query", [S, D], F32, kind="ExternalInput").ap(),
        "key_value": nc.dram_tensor("key_value", [S, D], F32, kind="ExternalInput").ap(),
        "Wq": nc.dram_tensor("Wq", [D, H * DK], F32, kind="ExternalInput").ap(),
        "Wk": nc.dram_tensor("Wk", [D, H * DK], F32, kind="ExternalInput").ap(),
        "Wv": nc.dram_tensor("Wv", [D, H * DV], F32, kind="ExternalInput").ap(),
        "Wo": nc.dram_tensor("Wo", [H * DV, D], F32, kind="ExternalInput").ap(),
        "bo": nc.dram_tensor("bo", [1, D], F32, kind="ExternalInput").ap(),
        "out": nc.dram_tensor("out", [S, D], F32, kind="ExternalOutput").ap(),
    }
    with tile.TileContext(nc) as tc:
        _emit(tc, aps)
    nc.compile()
    _CACHE["nc"] = nc
    return nc


LAST_RESULT = None


def kernel(query, key_value, Wq, Wk, Wv, Wo, bo):
    global LAST_RESULT
    nc = _build()
    query = np.ascontiguousarray(np.asarray(query, dtype=np.float32))
    key_value = np.ascontiguousarray(np.asarray(key_value, dtype=np.float32))
    shared = {
        "Wq": np.ascontiguousarray(np.asarray(Wq, dtype=np.float32)),
        "Wk": np.ascontiguousarray(np.asarray(Wk, dtype=np.float32)),
        "Wv": np.ascontiguousarray(np.asarray(Wv, dtype=np.float32)),
        "Wo": np.ascontiguousarray(np.asarray(Wo, dtype=np.float32)),
        "bo": np.ascontiguousarray(np.asarray(bo, dtype=np.float32)).reshape(1, D),
    }
    in_maps = [
        {"query": query[i], "key_value": key_value[i], **shared} for i in range(B)
    ]
    res = run_bass_kernel_spmd(
        nc, in_maps, core_ids=list(range(B)),
        trace=bool(int(os.environ.get("KERNEL_TRACE", "0"))),
    )
    LAST_RESULT = res
    return np.stack([r["out"] for r in res.results]).astype(np.float32)


if __name__ == "__main__":
    rng = np.random.default_rng(0)
    inputs = {
        "query": rng.standard_normal((B, S, D), dtype=np.float32),
        "key_value": rng.standard_normal((B, S, D), dtype=np.float32),
        "Wq": (rng.random((D, H * DK), dtype=np.float32) - 0.5) / 16.0,
        "Wk": (rng.random((D, H * DK), dtype=np.float32) - 0.5) / 16.0,
        "Wv": (rng.random((D, H * DV), dtype=np.float32) - 0.5) / 16.0,
        "Wo": (rng.random((H * DV, D), dtype=np.float32) - 0.5) / 16.0,
        "bo": (rng.random(D, dtype=np.float32) - 0.5) / 16.0,
    }
    y = kernel(**inputs)
    print("kernel out", y.shape, y.dtype, float(np.abs(y).max()))
